# revision 1
# baseline (speedup 1.0000x reference)
"""Trainium2 Bass kernel for CrossAttentionGCN (2-layer GCN per graph + cross-graph
MHA + 128x50000 output linear), distributed over 8 NeuronCores.

Sharding: core c handles graph c//2 and destination-node half c%2.

Design (v2): the GCN aggregation is SWDGE-gather bound (~8ns per edge index on
the Q7 descriptor generators), so everything else is arranged to hide under it:
- Nodes are permuted per graph (snake deal by in-degree) so the 49 dst windows
  of 512 nodes carry near-equal edge counts across windows AND cores; gather
  index padding drops to a few percent.
- The whole gather/matmul path runs in fp16: node tables are [50176, 128] fp16
  (256B rows = the SWDGE gather minimum); layer-1 features (64) zero-padded.
  The layer-1 table (x * dinv[src], permuted) is precomputed on the host.
- Per (512-window, src-half) one dma_gather (~4200 idx, the measured SWDGE
  sweet spot) fetches edge source rows; one-hot dst selectors (fp16 is_equal
  vs iota512) and PE matmuls reduce into a [feat, 512] fp32 PSUM tile.
  Self-loops use static identity selectors on the window's own rows (no
  gather). The W projection contracts the feature axis and transposes back.
- Layer-1 epilogue writes prescaled relu(h1)*dinv rows (fp16) into 4 chunk
  buffers; chunked pair AllGathers assemble the layer-2 table overlapped
  under the remaining layer-1 gathers.
- Pooled [B,H] embeddings are AllGathered 8-way; every core runs the tiny MHA
  and computes its own 6250-column slice of the output linear.
"""

import sys
import time

sys.path.insert(0, "/opt/trn_rl_repo")

import numpy as np
import ml_dtypes

import concourse.bass as bass
import concourse.bacc as bacc
import concourse.tile as tile
import concourse.mybir as mybir
from concourse.bass_utils import run_bass_kernel_spmd

dt = mybir.dt
NCORES = 8
P = 128
F16 = np.float16
NCHUNK = 4


class Cfg:
    def __init__(self, N=50000, E=800000, B=32, F=64, H=128, G=4):
        self.N, self.E, self.B, self.F, self.H, self.G = N, E, B, F, H, G
        self.HALF = 25088            # padded half size (196*128 = 49*512)
        self.NW = 49                 # 512-dst windows per half
        self.WSZ = 512
        self.NSUB = 4                # 128-subwindows per 512-window
        self.NPAD = 2 * self.HALF    # 50176 padded node ids per graph
        self.NCOLS = N // NCORES
        assert N % NCORES == 0
        # chunk boundaries (in windows) for the h1 exchange; tiny last chunk
        # so the L1->L2 dependency chain through the final collective is short
        self.CHW = [0, 16, 32, 48, self.NW]


def _permute_from_deg(deg_padded, cfg):
    """Snake-deal nodes (sorted by in-degree desc) into 98 bins of 512.
    Returns pi[padded old id] -> new id. Bin b -> half b//49, window b%49."""
    c = cfg
    nbins = 2 * c.NW
    order = np.argsort(-deg_padded, kind="stable")
    nb = np.empty(c.NPAD, np.int64)
    slot = np.empty(c.NPAD, np.int64)
    rounds = c.NPAD // nbins
    fwd = np.arange(nbins)
    seq = np.empty((rounds, nbins), np.int64)
    seq[0::2] = fwd
    seq[1::2] = fwd[::-1]
    pi = np.empty(c.NPAD, np.int64)
    nb[order] = seq.reshape(-1)
    slot[order] = np.repeat(np.arange(rounds), nbins)
    half = nb // c.NW
    w = nb % c.NW
    pi = half * c.HALF + w * c.WSZ + slot
    return pi


def host_prep(inputs, cfg):
    c = cfg
    x = np.asarray(inputs["x"], np.float32)
    ei = np.asarray(inputs["edge_index"]).astype(np.int64)
    batch = np.asarray(inputs["batch"]).astype(np.int64)
    W1 = np.asarray(inputs["W1"], np.float32)
    b1 = np.asarray(inputs["b1"], np.float32)
    W2 = np.asarray(inputs["W2"], np.float32)
    b2 = np.asarray(inputs["b2"], np.float32)
    in_proj_w = np.asarray(inputs["in_proj_w"], np.float32)
    in_proj_b = np.asarray(inputs["in_proj_b"], np.float32)
    out_proj_w = np.asarray(inputs["out_proj_w"], np.float32)
    out_proj_b = np.asarray(inputs["out_proj_b"], np.float32)
    lin_w = np.asarray(inputs["lin_w"], np.float32)
    lin_b = np.asarray(inputs["lin_b"], np.float32)

    G, N, B, H, F = c.G, c.N, c.B, c.H, c.F
    HALF, NW, WSZ = c.HALF, c.NW, c.WSZ

    # ---- per-graph: permutation, dinv, permuted tables ----
    g_pi, g_dinv_p, g_xhat, g_batch_p = [], [], [], []
    for g in range(G):
        row, col = ei[g, 0], ei[g, 1]
        deg = np.bincount(col, minlength=N).astype(np.float32) + 1.0
        dinv = (1.0 / np.sqrt(deg)).astype(np.float32)
        deg_in = np.zeros(c.NPAD, np.int64)
        np.add.at(deg_in, col, 1)
        pi = _permute_from_deg(deg_in, c)
        dinv_p = np.ones(c.NPAD, np.float32)
        dinv_p[pi[:N]] = dinv
        xhat = np.zeros((c.NPAD, H), F16)
        xhat[pi[:N], :F] = (x[g] * dinv[:, None]).astype(F16)
        batch_p = np.full(c.NPAD, 1000.0, np.float32)
        batch_p[pi[:N]] = batch[g].astype(np.float32)
        g_pi.append(pi)
        g_dinv_p.append(dinv_p)
        g_xhat.append(xhat)
        g_batch_p.append(batch_p)

    # ---- per-core edge slotting; shared K arrays ----
    per_core = []
    cnts = np.zeros((NCORES, NW, 2), np.int64)
    for core in range(NCORES):
        g, h = core // 2, core % 2
        row, col = ei[g, 0], ei[g, 1]
        pi = g_pi[g]
        sp, dp_ = pi[row], pi[col]
        m = (dp_ >= h * HALF) & (dp_ < (h + 1) * HALF)
        s, d = sp[m], dp_[m] - h * HALF
        w = d >> 9
        grp = (s >= HALF).astype(np.int64)
        sl = s - grp * HALF
        order = np.lexsort((d, grp, w))
        s, d, w, grp, sl = s[order], d[order], w[order], grp[order], sl[order]
        for gg in (0, 1):
            cnts[core, :, gg] = np.bincount(w[grp == gg], minlength=NW)
        per_core.append((sl, d, w, grp))

    K = np.maximum(-(-cnts.max(axis=0) // P), 1)  # [NW, 2] blocks
    KTOT = K.sum(axis=1)
    OFFB = np.concatenate([[0], np.cumsum(KTOT)]).astype(int)
    TOTB = int(OFFB[-1])

    # ---- MHA / linear constants (as baseline) ----
    linwT = np.ascontiguousarray(lin_w.T)
    inwT = np.ascontiguousarray(in_proj_w.T).astype(np.float32)
    HDs = np.sqrt(H // 8)
    inwT[:, :H] *= 1.0 / HDs
    bq = np.ascontiguousarray((in_proj_b[:H] / HDs).reshape(8, 16).T).astype(np.float32)
    bk = np.ascontiguousarray(in_proj_b[H:2 * H].reshape(8, 16).T).astype(np.float32)
    bv = in_proj_b[2 * H:].astype(np.float32)[:, None]
    outwT = np.ascontiguousarray(
        out_proj_w.T.reshape(8, 16, H).transpose(1, 0, 2).reshape(16, 8 * H)
    ).astype(np.float32)
    outb = out_proj_b.astype(np.float32)[:, None]
    ident = np.eye(P, dtype=np.float32)
    gb = np.arange(P)
    mask = np.where((gb[:, None] % B) == (gb[None, :] % B), 0.0, -30000.0).astype(np.float32)
    cntb = np.zeros((G, B), np.float32)
    for g in range(G):
        cntb[g] = np.bincount(batch[g], minlength=B).astype(np.float32)
    invc = np.where(cntb > 0, 1.0 / np.maximum(cntb, 1.0), 0.0).reshape(P, 1).astype(np.float32)
    ones1 = np.ones((1, 4), np.float32)

    iota512 = np.broadcast_to(np.arange(WSZ).astype(F16), (P, WSZ)).copy()
    iotaB = np.broadcast_to(np.arange(B).astype(F16), (P, B)).copy()
    identsel = np.zeros((P, c.NSUB * WSZ), F16)
    for sub in range(c.NSUB):
        identsel[np.arange(P), sub * WSZ + sub * P + np.arange(P)] = 1.0

    b1bc = np.broadcast_to(b1, (P, H)).astype(np.float32).copy()
    b2bc = np.broadcast_to(b2, (P, H)).astype(np.float32).copy()
    W1h = W1.astype(F16)
    W2h = W2.astype(F16)

    in_maps = []
    for core in range(NCORES):
        g, h = core // 2, core % 2
        sl, d, w, grp = per_core[core]

        idx = np.zeros((P, TOTB * 8), np.int16)
        dloc = np.full((P, TOTB), 2000.0, F16)
        for wi in range(NW):
            ob = int(OFFB[wi])
            for gg in (0, 1):
                kwg = int(K[wi, gg])
                o = ob + (int(K[wi, 0]) if gg else 0)
                mm = (w == wi) & (grp == gg)
                vals = sl[mm]
                dls = d[mm] & (WSZ - 1)
                slots = kwg * P
                sw = np.zeros(slots, np.int64)
                dw = np.full(slots, 2000, np.int64)
                sw[:len(vals)] = vals
                dw[:len(vals)] = dls
                wrap = sw.reshape(kwg * 8, 16).T.astype(np.int16)
                idx[:, o * 8:(o + kwg) * 8] = np.tile(wrap, (8, 1))
                dloc[:, o:o + kwg] = dw.reshape(kwg, P).T.astype(F16)

        dinv_half = g_dinv_p[g][h * HALF:(h + 1) * HALF]
        dinvd = np.ascontiguousarray(
            dinv_half.reshape(NW * c.NSUB, P).T).astype(np.float32)
        bloc = g_batch_p[g][h * HALF:(h + 1) * HALF]
        batchw = np.ascontiguousarray(
            bloc.reshape(NW * c.NSUB, P).T).astype(F16)
        tab1own = np.ascontiguousarray(
            g_xhat[g][h * HALF:(h + 1) * HALF]).astype(F16)

        in_maps.append(dict(
            tab1=g_xhat[g], tab1own=tab1own, idx=idx, dloc=dloc,
            dinvd=dinvd, batchw=batchw,
            W1h=W1h, W2h=W2h, b1bc=b1bc, b2bc=b2bc,
            iota512=iota512, iotaB=iotaB, identsel=identsel,
            ident=ident, mask=mask, invcnt=invc,
            inwT=inwT, bq=bq, bk=bk, bv=bv, outwT=outwT, outb=outb,
            linwT=np.ascontiguousarray(linwT[:, core * c.NCOLS:(core + 1) * c.NCOLS]),
            linb=lin_b[None, core * c.NCOLS:(core + 1) * c.NCOLS].astype(np.float32),
            ones1=ones1,
        ))

    meta = dict(K=K.astype(int), OFFB=OFFB, TOTB=TOTB)
    return in_maps, meta


def build_nc(cfg, meta, debug=False):
    c = cfg
    K, OFFB, TOTB = meta["K"], meta["OFFB"], meta["TOTB"]
    H, F, B, NW, WSZ, NSUB = c.H, c.F, c.B, c.NW, c.WSZ, c.NSUB
    HALF, NPAD = c.HALF, c.NPAD
    f32, f16, i16 = dt.float32, dt.float16, dt.int16
    AF = mybir.ActivationFunctionType
    OP = mybir.AluOpType

    nc = bacc.Bacc("TRN2", target_bir_lowering=False, debug=False,
                   enable_asserts=False, num_devices=NCORES)

    tab1 = nc.dram_tensor("tab1", [NPAD, H], f16, kind="ExternalInput")
    tab1own_t = nc.dram_tensor("tab1own", [HALF, H], f16, kind="ExternalInput")
    idx_t = nc.dram_tensor("idx", [P, TOTB * 8], i16, kind="ExternalInput")
    dloc_t = nc.dram_tensor("dloc", [P, TOTB], f16, kind="ExternalInput")
    dinvd_t = nc.dram_tensor("dinvd", [P, NW * NSUB], f32, kind="ExternalInput")
    batchw_t = nc.dram_tensor("batchw", [P, NW * NSUB], f16, kind="ExternalInput")
    W1h_t = nc.dram_tensor("W1h", [F, H], f16, kind="ExternalInput")
    W2h_t = nc.dram_tensor("W2h", [H, H], f16, kind="ExternalInput")
    b1bc_t = nc.dram_tensor("b1bc", [P, H], f32, kind="ExternalInput")
    b2bc_t = nc.dram_tensor("b2bc", [P, H], f32, kind="ExternalInput")
    iota512_t = nc.dram_tensor("iota512", [P, WSZ], f16, kind="ExternalInput")
    iotaB_t = nc.dram_tensor("iotaB", [P, B], f16, kind="ExternalInput")
    identsel_t = nc.dram_tensor("identsel", [P, NSUB * WSZ], f16, kind="ExternalInput")
    ident_t = nc.dram_tensor("ident", [P, P], f32, kind="ExternalInput")
    mask_t = nc.dram_tensor("mask", [P, P], f32, kind="ExternalInput")
    invcnt_t = nc.dram_tensor("invcnt", [P, 1], f32, kind="ExternalInput")
    inwT_t = nc.dram_tensor("inwT", [H, 3 * H], f32, kind="ExternalInput")
    bq_t = nc.dram_tensor("bq", [16, 8], f32, kind="ExternalInput")
    bk_t = nc.dram_tensor("bk", [16, 8], f32, kind="ExternalInput")
    bv_t = nc.dram_tensor("bv", [H, 1], f32, kind="ExternalInput")
    outwT_t = nc.dram_tensor("outwT", [16, 8 * H], f32, kind="ExternalInput")
    outb_t = nc.dram_tensor("outb", [H, 1], f32, kind="ExternalInput")
    linwT_t = nc.dram_tensor("linwT", [H, c.NCOLS], f32, kind="ExternalInput")
    linb_t = nc.dram_tensor("linb", [1, c.NCOLS], f32, kind="ExternalInput")
    ones1_t = nc.dram_tensor("ones1", [1, 4], f32, kind="ExternalInput")
    out = nc.dram_tensor("out", [4, c.NCOLS], f32, kind="ExternalOutput")
    if debug:
        dbg_tab2 = nc.dram_tensor("dbg_tab2", [NPAD, H], f32, kind="ExternalOutput")
        dbg_pool = nc.dram_tensor("dbg_pool", [NCORES * B, H], f32,
                                  kind="ExternalOutput")

    kmaxT = int((K[:, 0] + K[:, 1]).max())
    CHW = c.CHW

    with tile.TileContext(nc) as tc:
        with tc.tile_pool(name="consts", bufs=1) as cp, \
             tc.tile_pool(name="dram", bufs=1, space="DRAM") as dp:

            def load_const(src, shape, dtype):
                t = cp.tile(shape, dtype, tag=src.name)
                nc.sync.dma_start(out=t[:], in_=src[tuple(slice(0, s) for s in shape)])
                return t

            iota512_sb = load_const(iota512_t, [P, WSZ], f16)
            iotaB_sb = load_const(iotaB_t, [P, B], f16)
            identsel_sb = load_const(identsel_t, [P, NSUB * WSZ], f16)
            dinvd_sb = load_const(dinvd_t, [P, NW * NSUB], f32)
            batchw_sb = load_const(batchw_t, [P, NW * NSUB], f16)
            W1_sb = load_const(W1h_t, [F, H], f16)
            W2_sb = load_const(W2h_t, [H, H], f16)
            b1_sb = load_const(b1bc_t, [P, H], f32)
            b2_sb = load_const(b2bc_t, [P, H], f32)

            # layer-2 table: 4 per-chunk own buffers + assembled full table
            tab2own = [dp.tile([(CHW[i + 1] - CHW[i]) * WSZ, H], f16,
                               name=f"tab2own{i}", tag=f"tab2own{i}")
                       for i in range(NCHUNK)]
            tab2tmp = [dp.tile([2 * (CHW[i + 1] - CHW[i]) * WSZ, H], f16,
                               name=f"tab2tmp{i}", tag=f"tab2tmp{i}")
                       for i in range(NCHUNK)]
            tab2 = dp.tile([NPAD, H], f16, tag="tab2full")
            pool_in_t = dp.tile([B, H], f32, tag="pool_in")
            pool_all_t = dp.tile([NCORES * B, H], f32, tag="pool_all")

            with tc.tile_pool(name="mw", bufs=3) as mwp, \
                 tc.tile_pool(name="gath", bufs=3) as gp, \
                 tc.tile_pool(name="selfw", bufs=3) as sfp, \
                 tc.tile_pool(name="sel", bufs=6) as selp, \
                 tc.tile_pool(name="ep", bufs=4) as epp, \
                 tc.tile_pool(name="psA", bufs=2, space="PSUM") as psA, \
                 tc.tile_pool(name="psB", bufs=2, space="PSUM") as psB, \
                 tc.tile_pool(name="psPool", bufs=1, space="PSUM") as psP:

                pool_ps = psP.tile([B, H], f32, tag="pool")


                def chunk_of(w):
                    for i in range(NCHUNK):
                        if CHW[i] <= w < CHW[i + 1]:
                            return i, w - CHW[i]
                    raise AssertionError

                def gcn_window(layer, w):
                    feat = F if layer == 1 else H
                    Wmat = W1_sb if layer == 1 else W2_sb
                    bbc = b1_sb if layer == 1 else b2_sb
                    kA, kB = int(K[w, 0]), int(K[w, 1])
                    kt = kA + kB
                    ob = int(OFFB[w])
                    idx_sb = mwp.tile([P, kmaxT * 8], i16, tag="idx")
                    nc.sync.dma_start(out=idx_sb[:, :kt * 8],
                                      in_=idx_t[:, ob * 8:(ob + kt) * 8])
                    dloc_sb = mwp.tile([P, kmaxT], f16, tag="dloc")
                    nc.sync.dma_start(out=dloc_sb[:, :kt],
                                      in_=dloc_t[:, ob:ob + kt])
                    selfw = sfp.tile([P, NSUB * H], f16, tag="selfw")
                    if layer == 1:
                        nc.sync.dma_start(
                            out=selfw[:].rearrange("p (s f) -> p s f", f=H),
                            in_=tab1own_t[w * WSZ:(w + 1) * WSZ, :].rearrange(
                                "(s p) f -> p s f", p=P))
                    else:
                        ci, wloc = chunk_of(w)
                        nc.sync.dma_start(
                            out=selfw[:].rearrange("p (s f) -> p s f", f=H),
                            in_=tab2own[ci][wloc * WSZ:(wloc + 1) * WSZ, :].rearrange(
                                "(s p) f -> p s f", p=P))

                    table = tab1 if layer == 1 else tab2
                    g = gp.tile([P, kmaxT * H], f16, tag="g")
                    nc.gpsimd.dma_gather(
                        out_ap=g[:, :kA * H].rearrange("p (k f) -> p k f", f=H),
                        in_ap=table[0:HALF, :],
                        idxs_ap=idx_sb[:, :kA * 8],
                        num_idxs=kA * P, num_idxs_reg=kA * P,
                        elem_size=H, single_packet=False)
                    nc.gpsimd.dma_gather(
                        out_ap=g[:, kA * H:kt * H].rearrange("p (k f) -> p k f", f=H),
                        in_ap=table[HALF:NPAD, :],
                        idxs_ap=idx_sb[:, kA * 8:kt * 8],
                        num_idxs=kB * P, num_idxs_reg=kB * P,
                        elem_size=H, single_packet=False)

                    ps = psA.tile([feat, WSZ], f32, tag="agg")
                    for j in range(kt):
                        sel = selp.tile([P, WSZ], f16, tag="sel")
                        nc.vector.tensor_tensor(
                            out=sel[:],
                            in0=dloc_sb[:, j:j + 1].to_broadcast([P, WSZ]),
                            in1=iota512_sb[:],
                            op=OP.is_equal)
                        nc.tensor.matmul(
                            out=ps[:], lhsT=g[:, j * H:j * H + feat],
                            rhs=sel[:], start=(j == 0), stop=False)
                    for sub in range(NSUB):
                        nc.tensor.matmul(
                            out=ps[:],
                            lhsT=selfw[:, sub * H:sub * H + feat],
                            rhs=identsel_sb[:, sub * WSZ:(sub + 1) * WSZ],
                            start=False, stop=(sub == NSUB - 1))

                    aT = epp.tile([feat, WSZ], f16, tag="aT")
                    nc.vector.tensor_copy(out=aT[:], in_=ps[:])
                    for sub in range(NSUB):
                        wc = w * NSUB + sub
                        ps2 = psB.tile([P, H], f32, tag="proj")
                        nc.tensor.matmul(out=ps2[:],
                                         lhsT=aT[:, sub * P:(sub + 1) * P],
                                         rhs=Wmat[:], start=True, stop=True)
                        t1 = epp.tile([P, H], f32, tag="t1")
                        nc.vector.tensor_tensor(
                            out=t1[:], in0=ps2[:],
                            in1=dinvd_sb[:, wc:wc + 1].to_broadcast([P, H]),
                            op=OP.mult)
                        nc.vector.tensor_tensor(out=t1[:], in0=t1[:], in1=bbc[:],
                                                op=OP.add)
                        if layer == 1:
                            hw = epp.tile([P, H], f16, tag="hw1")
                            nc.scalar.activation(out=hw[:], in_=t1[:], func=AF.Relu,
                                                 scale=dinvd_sb[:, wc:wc + 1])
                            ci, wloc = chunk_of(w)
                            nc.sync.dma_start(
                                out=tab2own[ci][wloc * WSZ + sub * P:
                                                wloc * WSZ + (sub + 1) * P, :],
                                in_=hw[:])
                        else:
                            hw = epp.tile([P, H], f16, tag="hw2")
                            nc.scalar.activation(out=hw[:], in_=t1[:], func=AF.Relu)
                            poolsel = selp.tile([P, B], f16, tag="poolsel")
                            nc.vector.tensor_tensor(
                                out=poolsel[:],
                                in0=batchw_sb[:, wc:wc + 1].to_broadcast([P, B]),
                                in1=iotaB_sb[:], op=OP.is_equal)
                            nc.tensor.matmul(out=pool_ps[:], lhsT=poolsel[:],
                                             rhs=hw[:], start=(wc == 0),
                                             stop=(wc == NW * NSUB - 1))

                # ---- layer 1 with chunked h1 exchange ----
                for w in range(NW):
                    gcn_window(1, w)
                    for ci in range(NCHUNK):
                        if w == CHW[ci + 1] - 1:
                            lo = CHW[ci] * WSZ
                            hi = CHW[ci + 1] * WSZ
                            rows = hi - lo
                            nc.gpsimd.collective_compute(
                                "AllGather", OP.bypass,
                                replica_groups=[[0, 1], [2, 3], [4, 5], [6, 7]],
                                ins=[tab2own[ci][:, :]],
                                outs=[tab2tmp[ci][:, :]])
                            nc.sync.dma_start(out=tab2[lo:hi, :],
                                              in_=tab2tmp[ci][0:rows, :])
                            nc.sync.dma_start(out=tab2[HALF + lo:HALF + hi, :],
                                              in_=tab2tmp[ci][rows:2 * rows, :])
                # ---- layer 2 ----
                for w in range(NW):
                    gcn_window(2, w)

                pool_sb = epp.tile([B, H], f32, tag="poolsb")
                nc.vector.tensor_copy(out=pool_sb[:], in_=pool_ps[:])
                nc.sync.dma_start(out=pool_in_t[:], in_=pool_sb[:])

            nc.gpsimd.collective_compute(
                "AllGather", OP.bypass,
                replica_groups=[list(range(NCORES))],
                ins=[pool_in_t.opt()], outs=[pool_all_t.opt()])
            if debug:
                nc.sync.dma_start(out=dbg_pool[:, :], in_=pool_all_t[:, :])

            # ---- MHA + output linear (baseline structure, fp32) ----
            with tc.tile_pool(name="mha", bufs=1) as mh, \
                 tc.tile_pool(name="mmps", bufs=1, space="PSUM") as mmps, \
                 tc.tile_pool(name="sps", bufs=1, space="PSUM") as sps, \
                 tc.tile_pool(name="fin", bufs=2) as fp, \
                 tc.tile_pool(name="finps", bufs=2, space="PSUM") as fps:

                ident_sb = mh.tile([P, P], f32, tag="ident")
                nc.sync.dma_start(out=ident_sb[:], in_=ident_t[:, :])
                mask_sb = mh.tile([P, P], f32, tag="mask")
                nc.sync.dma_start(out=mask_sb[:], in_=mask_t[:, :])
                invc_sb = mh.tile([P, 1], f32, tag="invc")
                nc.sync.dma_start(out=invc_sb[:], in_=invcnt_t[:, :])
                inwT_sb = mh.tile([H, 3 * H], f32, tag="inwT")
                nc.sync.dma_start(out=inwT_sb[:], in_=inwT_t[:, :])
                bq_sb = mh.tile([16, 8], f32, tag="bq")
                nc.sync.dma_start(out=bq_sb[:], in_=bq_t[:, :])
                bk_sb = mh.tile([16, 8], f32, tag="bk")
                nc.sync.dma_start(out=bk_sb[:], in_=bk_t[:, :])
                bv_sb = mh.tile([H, 1], f32, tag="bv")
                nc.sync.dma_start(out=bv_sb[:], in_=bv_t[:, :])
                outwT_sb = mh.tile([16, 8 * H], f32, tag="outwT")
                nc.sync.dma_start(out=outwT_sb[:], in_=outwT_t[:, :])
                outb_sb = mh.tile([H, 1], f32, tag="outb")
                nc.sync.dma_start(out=outb_sb[:], in_=outb_t[:, :])

                ev = mh.tile([P, H], f32, tag="ev")
                od = mh.tile([P, H], f32, tag="od")
                for g4 in range(4):
                    nc.sync.dma_start(out=ev[g4 * B:(g4 + 1) * B, :],
                                      in_=pool_all_t[g4 * 2 * B:g4 * 2 * B + B, :])
                    nc.sync.dma_start(out=od[g4 * B:(g4 + 1) * B, :],
                                      in_=pool_all_t[g4 * 2 * B + B:(g4 + 1) * 2 * B, :])
                emb = mh.tile([P, H], f32, tag="emb")
                nc.vector.tensor_tensor(out=emb[:], in0=ev[:], in1=od[:], op=OP.add)
                nc.vector.tensor_tensor(
                    out=emb[:], in0=emb[:],
                    in1=invc_sb[:, 0:1].to_broadcast([P, H]), op=OP.mult)

                pt = mmps.tile([P, P], f32, tag="mm")
                nc.tensor.transpose(out=pt[:], in_=emb[:], identity=ident_sb[:])
                embT = mh.tile([P, P], f32, tag="embT")
                nc.vector.tensor_copy(out=embT[:], in_=pt[:])

                HD = 16

                def proj2(c0, bias_sb, tag):
                    pp = mmps.tile([16, 8 * P], f32, tag="mm2")
                    for hh in range(8):
                        nc.tensor.matmul(
                            out=pp[:, hh * P:(hh + 1) * P],
                            lhsT=inwT_sb[:, c0 + hh * HD:c0 + (hh + 1) * HD],
                            rhs=embT[:], start=True, stop=True)
                    o = mh.tile([16, 8 * P], f32, tag=tag)
                    nc.vector.tensor_tensor(
                        out=o[:].rearrange("p (h d) -> p h d", d=P),
                        in0=pp[:].rearrange("p (h d) -> p h d", d=P),
                        in1=bias_sb[:, :, None].to_broadcast([16, 8, P]),
                        op=OP.add)
                    return o

                q2 = proj2(0, bq_sb, "q2")
                k2 = proj2(H, bk_sb, "k2")

                vp0 = mmps.tile([P, P], f32, tag="mm")
                nc.tensor.matmul(out=vp0[:], lhsT=inwT_sb[:, 2 * H:3 * H],
                                 rhs=embT[:], start=True, stop=True)
                vT = mh.tile([P, P], f32, tag="vT")
                nc.vector.tensor_tensor(
                    out=vT[:], in0=vp0[:],
                    in1=bv_sb[:, 0:1].to_broadcast([P, P]), op=OP.add)

                s_ps = sps.tile([P, 8 * P], f32, tag="s")
                for hh in range(8):
                    nc.tensor.matmul(out=s_ps[:, hh * P:(hh + 1) * P],
                                     lhsT=q2[:16, hh * P:(hh + 1) * P],
                                     rhs=k2[:16, hh * P:(hh + 1) * P],
                                     start=True, stop=True)
                s_sb = mh.tile([P, 8 * P], f32, tag="ssb")
                nc.vector.tensor_tensor(
                    out=s_sb[:].rearrange("p (h d) -> p h d", d=P),
                    in0=s_ps[:].rearrange("p (h d) -> p h d", d=P),
                    in1=mask_sb[:, None, :].to_broadcast([P, 8, P]), op=OP.add)
                e_sb = mh.tile([P, 8 * P], f32, tag="esb")
                nc.scalar.activation(out=e_sb[:], in_=s_sb[:], func=AF.Exp)
                den = mh.tile([P, 8], f32, tag="den")
                nc.vector.reduce_sum(out=den[:],
                                     in_=e_sb[:].rearrange("p (h d) -> p h d", d=P),
                                     axis=mybir.AxisListType.X)
                rden = mh.tile([P, 8], f32, tag="rden")
                nc.vector.reciprocal(out=rden[:], in_=den[:])
                attn = mh.tile([P, 8 * P], f32, tag="attn")
                nc.vector.tensor_tensor(
                    out=attn[:].rearrange("p (h d) -> p h d", d=P),
                    in0=e_sb[:].rearrange("p (h d) -> p h d", d=P),
                    in1=rden[:, :, None].to_broadcast([P, 8, P]), op=OP.mult)

                vp = mmps.tile([P, P], f32, tag="mm")
                nc.tensor.transpose(out=vp[:], in_=vT[:], identity=ident_sb[:])
                v_sb = mh.tile([P, P], f32, tag="vsb")
                nc.vector.tensor_copy(out=v_sb[:], in_=vp[:])

                ctx2_ps = mmps.tile([16, 8 * P], f32, tag="mm2")
                for hh in range(8):
                    ap_ps = mmps.tile([P, P], f32, tag="mm")
                    nc.tensor.transpose(out=ap_ps[:],
                                        in_=attn[:, hh * P:(hh + 1) * P],
                                        identity=ident_sb[:])
                    at_sb = mh.tile([P, P], f32, tag="atsb")
                    nc.vector.tensor_copy(out=at_sb[:], in_=ap_ps[:])
                    nc.tensor.matmul(out=ctx2_ps[:16, hh * P:(hh + 1) * P],
                                     lhsT=v_sb[:, hh * HD:(hh + 1) * HD],
                                     rhs=at_sb[:], start=True, stop=True)
                ctx2_sb = mh.tile([16, 8 * P], f32, tag="ctx2sb")
                nc.vector.tensor_copy(out=ctx2_sb[:], in_=ctx2_ps[:])

                ao_ps = mmps.tile([P, P], f32, tag="mm")
                for hh in range(8):
                    nc.tensor.matmul(out=ao_ps[:],
                                     lhsT=outwT_sb[:16, hh * H:(hh + 1) * H],
                                     rhs=ctx2_sb[:16, hh * P:(hh + 1) * P],
                                     start=(hh == 0), stop=(hh == 7))
                attT = mh.tile([P, P], f32, tag="attT")
                nc.vector.tensor_tensor(
                    out=attT[:], in0=ao_ps[:],
                    in1=outb_sb[:, 0:1].to_broadcast([P, P]), op=OP.add)

                pooledT_raw = mh.tile([P, 4], f32, tag="praw")
                nc.vector.reduce_sum(out=pooledT_raw[:],
                                     in_=attT[:].rearrange("p (g b) -> p g b", b=B),
                                     axis=mybir.AxisListType.X)
                pooledT = mh.tile([P, 4], f32, tag="pooledT")
                nc.scalar.activation(out=pooledT[:], in_=pooledT_raw[:],
                                     func=AF.Copy, scale=1.0 / B)

                linw_sb = mh.tile([H, c.NCOLS], f32, tag="linw")
                nc.sync.dma_start(out=linw_sb[:], in_=linwT_t[:, :])
                linb_sb = mh.tile([1, c.NCOLS], f32, tag="linb")
                nc.sync.dma_start(out=linb_sb[:], in_=linb_t[:, :])
                ones_sb = mh.tile([1, 4], f32, tag="ones")
                nc.sync.dma_start(out=ones_sb[:], in_=ones1_t[:, :])

                CH = 512
                for c0 in range(0, c.NCOLS, CH):
                    cw = min(CH, c.NCOLS - c0)
                    fps_t = fps.tile([4, CH], f32, tag="fin")
                    nc.tensor.matmul(out=fps_t[:, :cw], lhsT=pooledT[:, :4],
                                     rhs=linw_sb[:, c0:c0 + cw], start=True, stop=False)
                    nc.tensor.matmul(out=fps_t[:, :cw], lhsT=ones_sb[0:1, :4],
                                     rhs=linb_sb[0:1, c0:c0 + cw], start=False, stop=True)
                    ob = fp.tile([4, CH], f32, tag="ob")
                    nc.scalar.activation(out=ob[:, :cw], in_=fps_t[:, :cw],
                                         func=AF.Copy, scale=60.0, bias=50.0)
                    nc.sync.dma_start(out=out[0:4, c0:c0 + cw], in_=ob[:, :cw])

            if debug:
                nc.sync.dma_start(out=dbg_tab2[:, :], in_=tab2[:, :])

    nc.compile()
    return nc


def run_cfg(inputs, cfg, debug=False, want_results=False):
    in_maps, meta = host_prep(inputs, cfg)
    nc = build_nc(cfg, meta, debug=debug)
    last_err = None
    for attempt in range(3):
        try:
            res = run_bass_kernel_spmd(nc, in_maps, core_ids=list(range(NCORES)))
            break
        except Exception as e:  # transient NRT device recovery
            last_err = e
            time.sleep(5.0)
    else:
        raise last_err
    outp = np.empty((4, cfg.N), np.float32)
    for core in range(NCORES):
        outp[:, core * cfg.NCOLS:(core + 1) * cfg.NCOLS] = res.results[core]["out"]
    if want_results:
        return outp, res
    return outp


def kernel(**inputs) -> np.ndarray:
    return run_cfg(inputs, Cfg())

